# revision 39
# baseline (speedup 1.0000x reference)
"""Causal self-attention (GPT-style block) on 8 Trainium2 NeuronCores.

Sharding: tensor-parallel over heads (16 heads / 8 cores = 2 heads/core).
c_attn column-parallel from full x; attention fully local per core;
c_proj token-parallel after on-device AllToAll exchanges for batches
0..2, row-parallel for batch 3 (host sums the 8 partials).

Key structural choices (all matmuls contract over the partition dim;
matmul inputs are bf16 so every matmul runs at 1 cycle/row on the PE
regardless of free size):
- x is host-pretransposed/tiled to xp[tb, p, kt, s] (bf16) so stage 1
  needs no device transposes.
- q,k produced channel-major ([chan, tok]); v produced DIRECTLY
  token-major (stationary = x tile, moving = w_v) into vaug tiles
  [tok, v_h0(64) | 1 | v_h1(64) | 1]; the ones columns accumulate the
  softmax denominators inside the PV matmuls.
- b_q/b_k folded into the psum eviction (tensor_scalar_add); b_v is
  folded into b_proj ON THE HOST (b_v @ w_proj is a constant row).
- Scores computed transposed per 512-query block: S^T[key, q] with the
  2 heads packed in one [P, 2, 512] psum tile (row-tiled PE loads);
  exp is a single ACT instruction per key tile (strided AP covers both
  heads, including partial diagonal widths); causal mask applied
  multiplicatively to E on DVE.
- PV is TOKEN-major: out[q, chan] = sum_k E^T[k,q] v[k,chan] with
  stationary = 128x128 E tile, moving = vaug slice (65 free). K=128
  and out partitions = queries: half the PE cost of channel-major PV,
  and softmax normalization becomes a per-partition tensor_scalar_mul
  fused into the eviction. A PE transpose restores channel-major yT
  for the projection.
- c_proj: one AllToAll per batch 0..2 (the collective cost model has
  ~15us fixed overhead, so fewer+bigger wins); batch b's projection is
  emitted late in attention(b+1), hiding the collective latency.
- Stage 1 of batch b+1, ready projections, and b3's row-parallel
  partial projections are interleaved into attention's instruction
  stream via filler queues, so the PE never idles behind ACT's exp
  latency.
"""

import collections

import numpy as np

P = 128
B = 4
T = 2048
BT = B * T            # 8192 tokens
C = 1024
KT = C // P           # 8 contraction tiles of 128 input channels
NTB = BT // 512       # 16 token blocks of 512
HD = 64               # head dim
NQ = T // 512         # 4 query blocks per batch
NCORES = 8
TPC = 256             # tokens per core per exchanged batch

_CACHED = {}


def _build_nc():
    import concourse.mybir as mybir
    import concourse.tile as tile
    from concourse import bacc
    from concourse.masks import make_identity

    f32 = mybir.dt.float32
    bf16 = mybir.dt.bfloat16
    EXP = mybir.ActivationFunctionType.Exp

    nc = bacc.Bacc("TRN2", target_bir_lowering=False, debug=False,
                   num_devices=NCORES)

    xp = nc.dram_tensor("xp", [NTB, P, KT, 512], bf16, kind="ExternalInput")
    wq = nc.dram_tensor("wq", [P, KT, P], bf16, kind="ExternalInput")
    wk = nc.dram_tensor("wk", [P, KT, P], bf16, kind="ExternalInput")
    wv = nc.dram_tensor("wv", [P, KT, P], bf16, kind="ExternalInput")
    wp = nc.dram_tensor("wp", [P, KT, C], bf16, kind="ExternalInput")
    wpr = nc.dram_tensor("wpr", [P, C], bf16, kind="ExternalInput")
    bq = nc.dram_tensor("bq", [P, 1], f32, kind="ExternalInput")
    bk = nc.dram_tensor("bk", [P, 1], f32, kind="ExternalInput")
    # outputs: ypx[u] = batch u, this core's 256 tokens; ypl = batch 3
    # row-parallel partial over all 2048 tokens (host sums 8 cores)
    ypx = nc.dram_tensor("ypx", [3, TPC, C], bf16, kind="ExternalOutput")
    ypl = nc.dram_tensor("ypl", [T, C], bf16, kind="ExternalOutput")

    with tile.TileContext(nc) as tc:
        with (
            tc.tile_pool(name="const", bufs=1) as const,
            tc.tile_pool(name="xt", bufs=3) as xt_pool,
            tc.tile_pool(name="slab", bufs=2) as slab_pool,
            tc.tile_pool(name="e", bufs=4) as e_pool,
            tc.tile_pool(name="ytn", bufs=4) as ytn_pool,
            tc.tile_pool(name="nrm", bufs=8) as nrm_pool,
            tc.tile_pool(name="ob", bufs=4) as ob_pool,
            tc.tile_pool(name="yg", bufs=2) as yg_pool,
            tc.tile_pool(name="dram", bufs=1, space="DRAM") as dram_pool,
            tc.tile_pool(name="pss", bufs=2, space="PSUM") as pss_pool,
            tc.tile_pool(name="pvo", bufs=2, space="PSUM") as pvo_pool,
            tc.tile_pool(name="ps1", bufs=2, space="PSUM") as ps1_pool,
        ):
            g_in = [dram_pool.tile([NCORES, P, TPC], bf16,
                                   name=f"g_in{u}", tag=f"g_in{u}")
                    for u in range(3)]
            g_out = [dram_pool.tile([NCORES, P, TPC], bf16,
                                    name=f"g_out{u}", tag=f"g_out{u}")
                     for u in range(3)]

            # --- constants / weights resident in SBUF ---
            wq_sb = const.tile([P, KT, P], bf16)
            wk_sb = const.tile([P, KT, P], bf16)
            wv_sb = const.tile([P, KT, P], bf16)
            wp_sb = const.tile([P, KT, C], bf16)
            wpr_sb = const.tile([P, C], bf16)
            bq_sb = const.tile([P, 1], f32)
            bk_sb = const.tile([P, 1], f32)

            ident_f = const.tile([P, P], f32)
            make_identity(nc, ident_f[:])
            ident = const.tile([P, P], bf16)
            nc.vector.tensor_copy(ident[:], ident_f[:])

            # mask[p, s] = 1.0 if s >= p else 0.0 (keep q >= k)
            mask_f = const.tile([P, P], f32)
            nc.gpsimd.memset(mask_f[:], 1.0)
            nc.gpsimd.affine_select(
                out=mask_f[:],
                in_=mask_f[:],
                compare_op=mybir.AluOpType.is_ge,
                fill=0.0,
                base=0,
                pattern=[[1, P]],
                channel_multiplier=-1,
            )
            mask2 = const.tile([P, 2, P], bf16)
            nc.vector.tensor_copy(mask2[:, 0, :], mask_f[:])
            nc.vector.tensor_copy(mask2[:, 1, :], mask_f[:])

            # kt-0 slice first: the very first matmul only needs 32KB;
            # the bulk of wq follows the first x chunk (see load_xt)
            nc.sync.dma_start(wq_sb[:, 0:1, :], wq[:, 0:1, :])
            # w_proj loads deferred off the startup path; scalar queue so
            # they don't delay xp streaming on the sync queue
            wp_loaded = []

            def load_wp():
                if not wp_loaded:
                    # hold the 2MB load back so the scheduler can't hoist it
                    # over the startup xp chunks on the shared DMA engine
                    with tc.tile_wait_until(0.04):
                        nc.scalar.dma_start(wp_sb[:], wp[:])
                        nc.scalar.dma_start(wpr_sb[:], wpr[:])
                    wp_loaded.append(True)

            slabs = {}
            yTs = {}

            # ---------------- stage 1 (qkv) ----------------
            def stage1_units(b, startup=False):
                """Emission closures for batch b's qkv work, ordered so each
                x-block's DMA is issued ~4 units before its first use."""
                qT = slab_pool.tile([P, T], bf16, tag="qT", name=f"qT{b}")
                kT = slab_pool.tile([P, T], bf16, tag="kT", name=f"kT{b}")
                vaug = slab_pool.tile([P, T // P, 2 * HD + 2], bf16,
                                      tag="vaug", name=f"vaug{b}")
                slabs[b] = (qT, kT, vaug)

                def ones_cols():
                    nc.vector.memset(vaug[:, :, HD:HD + 1], 1.0)
                    nc.vector.memset(vaug[:, :, 2 * HD + 1:2 * HD + 2], 1.0)

                xts = {}

                def load_xt(lb, chunked=False):
                    def emit():
                        xt = xt_pool.tile([P, KT, 512], bf16, tag="xt",
                                          name=f"xt{b}_{lb}")
                        xts[lb] = xt
                        tb = b * NQ + lb
                        if chunked:
                            # front-load a tiny first chunk so the PE can
                            # start ~1us in; balance the rest against the
                            # per-DMA HWDGE overhead
                            nc.sync.dma_start(xt[:, 0:1, :], xp[tb, :, 0:1, :])
                            nc.sync.dma_start(wq_sb[:, 1:KT, :], wq[:, 1:KT, :])
                            nc.sync.dma_start(xt[:, 1:4, :], xp[tb, :, 1:4, :])
                            nc.sync.dma_start(wk_sb[:], wk[:])
                            nc.sync.dma_start(xt[:, 4:8, :], xp[tb, :, 4:8, :])
                            nc.sync.dma_start(wv_sb[:], wv[:])
                            nc.sync.dma_start(bq_sb[:], bq[:])
                            nc.sync.dma_start(bk_sb[:], bk[:])
                        else:
                            nc.sync.dma_start(xt[:], xp[tb])
                    return emit

                def qk_group(lb, w_sb, b_sb, dst):
                    def emit():
                        xt = xts[lb]
                        sl = slice(lb * 512, (lb + 1) * 512)
                        ps = ps1_pool.tile([P, 512], f32, tag="ps1",
                                           name=f"ps_qk{b}_{lb}")
                        for kt in range(KT):
                            nc.tensor.matmul(ps[:], w_sb[:, kt, :],
                                             xt[:, kt, :],
                                             start=(kt == 0),
                                             stop=(kt == KT - 1))
                        nc.vector.tensor_scalar_add(dst[:, sl], ps[:], b_sb[:])
                    return emit

                def v_group(lb, pair):
                    def emit():
                        xt = xts[lb]
                        psv = ps1_pool.tile([P, 512], f32, tag="ps1",
                                            name=f"ps_v{b}_{lb}_{pair}")
                        for t4 in (2 * pair, 2 * pair + 1):
                            off = t4 * P
                            tsl = slice(off, off + P)
                            for kt in range(KT):
                                nc.tensor.matmul(psv[:, tsl],
                                                 xt[:, kt, tsl],
                                                 wv_sb[:, kt, :],
                                                 start=(kt == 0),
                                                 stop=(kt == KT - 1))
                            j4 = lb * 4 + t4
                            nc.vector.tensor_copy(vaug[:, j4, 0:HD],
                                                  psv[:, off:off + HD])
                            nc.vector.tensor_copy(
                                vaug[:, j4, HD + 1:2 * HD + 1],
                                psv[:, off + HD:off + P])
                    return emit

                QK, VG = 1707, 854   # est PE ns per unit
                units = [(0, ones_cols),
                         (0, load_xt(0, chunked=startup)),
                         (QK, qk_group(0, wq_sb, bq_sb, qT)),
                         (0, load_xt(1)),
                         (QK, qk_group(0, wk_sb, bk_sb, kT)),
                         (VG, v_group(0, 0)), (VG, v_group(0, 1))]
                for lb in range(1, NQ):
                    units.append((QK, qk_group(lb, wq_sb, bq_sb, qT)))
                    if lb + 1 < NQ:
                        units.append((0, load_xt(lb + 1)))
                    units.append((QK, qk_group(lb, wk_sb, bk_sb, kT)))
                    units.append((VG, v_group(lb, 0)))
                    units.append((VG, v_group(lb, 1)))
                return units

            # ---------------- c_proj pieces ----------------
            def proj_unit(u, tok0, act_half=False):
                """One 128-token projection tile from exchanged batch u.
                act_half splits the eviction DVE/ACT -- only for units that
                run in batches where ACT is not the local metronome."""
                def emit():
                    yg = yg_tiles[u]
                    tsl = slice(tok0, tok0 + P)
                    pp0 = ps1_pool.tile([P, 512], f32, tag="ps1",
                                        name=f"pp0_{u}_{tok0}")
                    pp1 = ps1_pool.tile([P, 512], f32, tag="ps1",
                                        name=f"pp1_{u}_{tok0}")
                    for ct in range(KT):
                        nc.tensor.matmul(pp0[:], yg[:, ct, tsl],
                                         wp_sb[:, ct, 0:512],
                                         start=(ct == 0), stop=(ct == KT - 1))
                    for ct in range(KT):
                        nc.tensor.matmul(pp1[:], yg[:, ct, tsl],
                                         wp_sb[:, ct, 512:C],
                                         start=(ct == 0), stop=(ct == KT - 1))
                    ob = ob_pool.tile([P, C], bf16, tag="ob",
                                      name=f"ob_{u}_{tok0}")
                    nc.vector.tensor_copy(ob[:, 0:512], pp0[:])
                    if act_half:
                        nc.scalar.copy(ob[:, 512:C], pp1[:])
                    else:
                        nc.vector.tensor_copy(ob[:, 512:C], pp1[:])
                    out_dmas.append(
                        lambda: nc.scalar.dma_start(ypx[u, tsl, :], ob[:]))
                return emit

            def partial_unit(pt, tail=False):
                """Row-parallel partial proj for b3 tokens [128pt, +128)."""
                def emit():
                    yT3 = yTs[3]
                    ssl = slice(pt * P, (pt + 1) * P)
                    pp0 = ps1_pool.tile([P, 512], f32, tag="ps1",
                                        name=f"ppl0_{pt}")
                    pp1 = ps1_pool.tile([P, 512], f32, tag="ps1",
                                        name=f"ppl1_{pt}")
                    nc.tensor.matmul(pp0[:], yT3[:, ssl], wpr_sb[:, 0:512],
                                     start=True, stop=True)
                    nc.tensor.matmul(pp1[:], yT3[:, ssl], wpr_sb[:, 512:C],
                                     start=True, stop=True)
                    ob = ob_pool.tile([P, C], bf16, tag="ob",
                                      name=f"obl_{pt}")
                    nc.vector.tensor_copy(ob[:, 0:512], pp0[:])
                    if tail:
                        # ACT is idle at the drain; halve the DVE chain
                        nc.scalar.copy(ob[:, 512:C], pp1[:])
                    else:
                        nc.vector.tensor_copy(ob[:, 512:C], pp1[:])
                    out_dmas.append(
                        lambda: nc.scalar.dma_start(ypl[ssl, :], ob[:]))
                return emit

            yg_tiles = {}
            out_dmas = collections.deque()
            deferred = collections.deque()  # norm->transpose + exchanges

            def emit_exchange(u):
                """AllToAll batch u's yT; peer j gets this core's 2
                head-channels for peer j's 256 tokens."""
                yT_ = yTs[u]
                for j in range(NCORES):
                    # gpsimd queue: these wait on late yT regions; on the SP
                    # queue that wait would block the next batch's xt loads
                    nc.gpsimd.dma_start(g_in[u][j],
                                        yT_[:, j * TPC:(j + 1) * TPC])
                nc.gpsimd.collective_compute(
                    "AllToAll",
                    mybir.AluOpType.bypass,
                    replica_groups=[list(range(NCORES))],
                    ins=[g_in[u][:]],
                    outs=[g_out[u][:]],
                )
                # gather on the gpsimd queue: it naturally orders after the
                # collective without blocking any busy engine's queue
                yg = yg_pool.tile([P, NCORES, TPC], bf16, tag="yg",
                                  name=f"yg{u}")
                nc.gpsimd.dma_start(yg[:], g_out[u].rearrange("c p t -> p c t"))
                yg_tiles[u] = yg

            # ---------------- attention ----------------
            def attention(b, fillers, late2=(), late3=(), own=()):
                qT, kT, vaug = slabs[b]
                yT = slab_pool.tile([P, T], bf16, tag="yT", name=f"yT{b}")
                yTs[b] = yT
                fillers = collections.deque(fillers)
                late2 = collections.deque(late2)
                late3 = collections.deque(late3)
                own = collections.deque(own)  # (block, cost, closure)
                need = 0.0   # cumulative ACT-minus-PE ns not yet filled

                def pump(i, j_done, j_total):
                    nonlocal need
                    if deferred:
                        deferred.popleft()()
                    if out_dmas:
                        out_dmas.popleft()()
                    if own:
                        own.popleft()[2]()
                        return
                    if i >= 2 and late2 and (j_done % 2 == 0):
                        late2.popleft()[1]()
                        return
                    if i >= 3 and late3 and (j_done % 2 == 1):
                        late3.popleft()[1]()
                        return
                    rem_j = j_total - j_done
                    while fillers and len(fillers) >= rem_j:
                        fillers.popleft()[1]()
                    if fillers and (j_done % 2 == 0):
                        fillers.popleft()[1]()

                j_total = sum(4 * (i + 1) for i in range(NQ))
                j_done = 0
                for i in range(NQ):
                    # this batch's own stage-1 blocks <= i must be emitted
                    # before block i's scores touch them
                    while own and own[0][0] <= i:
                        _, cost, u = own.popleft()
                        u()
                        need -= max(cost, 400)
                    nj = 4 * (i + 1)
                    # 8 accumulation regions share 2 banks. The FIRST matmul
                    # into each bank (j=0, lqt 0/2, h0) uses start=True: its
                    # whole-zero-region lazy-clear zeroes all 4 regions of
                    # the bank at once; every other matmul accumulates.
                    pvo_a = pvo_pool.tile([P, 512], f32, tag="pvo",
                                          name=f"pvo_a{b}_{i}")
                    pvo_b = pvo_pool.tile([P, 512], f32, tag="pvo",
                                          name=f"pvo_b{b}_{i}")
                    pvs = (pvo_a, pvo_b)

                    def emit_s(j):
                        q0 = max(0, (j - 4 * i)) * P
                        jsl = slice(j * P, (j + 1) * P)
                        qsl = slice(i * 512 + q0, (i + 1) * 512)
                        psp = pss_pool.tile([P, 2, 512], f32, tag="pss",
                                            name=f"psp{b}_{i}_{j}")
                        nc.tensor.matmul(psp[:, 0, q0:512], kT[0:HD, jsl],
                                         qT[0:HD, qsl], start=True, stop=True,
                                         tile_position=(0, 0))
                        nc.tensor.matmul(psp[:, 1, q0:512], kT[HD:P, jsl],
                                         qT[HD:P, qsl], start=True, stop=True,
                                         tile_position=(HD, 0))
                        return psp

                    def emit_e(j, psp):
                        q0 = max(0, (j - 4 * i)) * P
                        ep = e_pool.tile([P, 2, 512], bf16, tag="e",
                                         name=f"ep{b}_{i}_{j}")
                        # one activation covers both heads, including the
                        # partial diagonal widths (strided [2, w] AP)
                        nc.scalar.activation(ep[:, :, q0:512],
                                             psp[:, :, q0:512], EXP,
                                             scale=0.125)
                        if j - 4 * i >= 0:
                            d = j - 4 * i
                            msl = slice(d * P, (d + 1) * P)
                            nc.vector.tensor_mul(ep[:, 0, msl], ep[:, 0, msl],
                                                 mask2[:, 0, :])
                            nc.vector.tensor_mul(ep[:, 1, msl], ep[:, 1, msl],
                                                 mask2[:, 1, :])
                        return ep

                    def emit_norm(b_, qt, psA, off):
                        rinv = nrm_pool.tile([P, 2], f32, tag="rinv",
                                             name=f"rinv{b_}_{qt}")
                        nc.vector.reciprocal(rinv[:, 0:1],
                                             psA[:, off + HD:off + HD + 1])
                        nc.vector.reciprocal(rinv[:, 1:2],
                                             psA[:, off + 129:off + 130])
                        ytn = ytn_pool.tile([P, P], bf16, tag="ytn",
                                            name=f"ytn{b_}_{qt}")
                        nc.vector.tensor_scalar_mul(
                            ytn[:, 0:HD], psA[:, off:off + HD], rinv[:, 0:1])
                        nc.vector.tensor_scalar_mul(
                            ytn[:, HD:P], psA[:, off + HD + 1:off + 129],
                            rinv[:, 1:2])

                        def finish():
                            psT = ps1_pool.tile([P, P], bf16, tag="ps1",
                                                name=f"psT{b_}_{qt}")
                            nc.tensor.transpose(psT[:], ytn[:], ident[:])
                            nc.vector.tensor_copy(
                                yT[:, qt * P:(qt + 1) * P], psT[:])
                        deferred.append(finish)

                    def emit_pv(j, ep):
                        d = max(0, j - 4 * i)
                        for lqt in range(d, 4):
                            qt = 4 * i + lqt
                            psA = pvs[lqt // 2]
                            off = (lqt % 2) * 256
                            esl = slice(lqt * P, (lqt + 1) * P)
                            sp = (j == qt)
                            st = (j == 0 and lqt in (0, 2))
                            nc.tensor.matmul(psA[:, off:off + HD + 1],
                                             ep[:, 0, esl],
                                             vaug[:, j, 0:HD + 1],
                                             start=st, stop=sp,
                                             skip_group_check=True)
                            nc.tensor.matmul(psA[:, off + HD + 1:off + 130],
                                             ep[:, 1, esl],
                                             vaug[:, j, HD + 1:2 * HD + 2],
                                             start=False, stop=sp,
                                             skip_group_check=True)
                            if sp:
                                emit_norm(b, qt, psA, off)

                    # software pipeline: S(j+1) issued before PV(j) so the
                    # PE never sits directly behind ACT's exp latency
                    psps = {0: emit_s(0)}
                    eps = {}
                    for j in range(nj):
                        eps[j] = emit_e(j, psps.pop(j))
                        if j + 1 < nj:
                            psps[j + 1] = emit_s(j + 1)
                        emit_pv(j, eps.pop(j))
                        j_done += 1
                        pump(i, j_done, j_total)
                while own:
                    own.popleft()[2]()
                for q in (late2, late3, fillers):
                    while q:
                        q.popleft()[1]()

            # ---------------- main schedule ----------------
            # Per-phase PE load balancing: each batch's attention carries
            # its OWN stage-1 blocks 1-3 (block i forced before query-block
            # i), while only block 0 of the NEXT batch is prefed late in the
            # current attention. This levels PE work across all 4 phases.
            BLK = [1, 1, 1, 1, 1, 2, 2, 2, 2, 2, 3, 3, 3, 3]

            def split_units(units):
                # pre: block 0 (7 units), own: blocks 1-3 folded into the
                # batch's own attention
                return (units[:7],
                        [(bi, c, u) for bi, (c, u) in zip(BLK, units[7:])])

            s0 = stage1_units(0, startup=True)
            pre0, own0 = split_units(s0)
            for _, u in pre0:
                u()

            PJ, PL = 3414, 854
            pre1, own1 = split_units(stage1_units(1))
            attention(0, [], late3=list(pre1), own=own0)
            deferred.append(lambda: emit_exchange(0))
            load_wp()

            pre2, own2 = split_units(stage1_units(2))
            attention(1, [], own=own1,
                      late3=[(PJ, proj_unit(0, 0, act_half=True))] + list(pre2))
            deferred.append(lambda: emit_exchange(1))

            # batch-0/1 proj units are ready well before b2/b3 start, so
            # they serve as base fillers there instead of crowding late3
            pre3, own3 = split_units(stage1_units(3))
            attention(2, [(PJ, proj_unit(0, 128, act_half=True))], own=own2,
                      late3=[(PJ, proj_unit(1, 0, act_half=True))] + list(pre3))
            deferred.append(lambda: emit_exchange(2))

            # b3: batch-1/2 proj + row-parallel partials as fillers
            late2 = [(PL, partial_unit(pt)) for pt in range(8)]
            late3 = ([(PL, partial_unit(pt)) for pt in range(8, 12)]
                     + [(PJ, proj_unit(2, 0)), (PJ, proj_unit(2, 128))]
                     + [(PL, partial_unit(12)), (PL, partial_unit(13)),
                        (PL, partial_unit(14))])
            attention(3, [(PJ, proj_unit(1, 128))], late2=late2, late3=late3,
                      own=own3)

            # tail: last partial tiles (tokens 1536:2048 of b3)
            while deferred:
                deferred.popleft()()
            for pt in range(15, 16):
                partial_unit(pt, tail=True)()
                while out_dmas:
                    out_dmas.popleft()()

    nc.compile()
    return nc


def _prep_inputs(x, w_attn, b_attn, w_proj):
    import ml_dtypes
    bf16 = ml_dtypes.bfloat16

    x = np.asarray(x, dtype=np.float32)
    w_attn = np.asarray(w_attn, dtype=np.float32)
    b_attn = np.asarray(b_attn, dtype=np.float32)
    w_proj = np.asarray(w_proj, dtype=np.float32)

    x_flat = x.reshape(BT, C)
    # xp[tb, p, kt, s] = x_flat[tb*512+s, kt*128+p]
    xp = np.ascontiguousarray(
        x_flat.T.reshape(KT, P, NTB, 512).transpose(2, 1, 0, 3)).astype(bf16)

    wp = np.ascontiguousarray(
        w_proj.reshape(KT, P, C).transpose(1, 0, 2)).astype(bf16)
    in_maps = []
    for c in range(NCORES):
        cols = slice(P * c, P * (c + 1))

        def wslice(off):
            w = w_attn[:, off + P * c: off + P * (c + 1)]   # [1024, 128]
            return np.ascontiguousarray(
                w.reshape(KT, P, P).transpose(1, 0, 2)).astype(bf16)

        in_maps.append({
            "xp": xp,
            "wq": wslice(0),
            "wk": wslice(C),
            "wv": wslice(2 * C),
            "wp": wp,
            "wpr": np.ascontiguousarray(w_proj[cols, :]).astype(bf16),
            "bq": np.ascontiguousarray(b_attn[cols]).reshape(P, 1),
            "bk": np.ascontiguousarray(
                b_attn[C + P * c: C + P * (c + 1)]).reshape(P, 1),
        })
    return in_maps


def kernel(x, w_attn, b_attn, w_proj, b_proj):
    from concourse.bass_utils import run_bass_kernel_spmd

    if "nc" not in _CACHED:
        _CACHED["nc"] = _build_nc()
    nc = _CACHED["nc"]

    in_maps = _prep_inputs(x, w_attn, b_attn, w_proj)
    res = run_bass_kernel_spmd(nc, in_maps, core_ids=list(range(NCORES)))

    w_proj = np.asarray(w_proj, dtype=np.float32)
    b_attn = np.asarray(b_attn, dtype=np.float32)
    y = np.empty((B, T, C), dtype=np.float32)
    for c in range(NCORES):
        r = res.results[c]
        for u in range(3):
            y[u, TPC * c:TPC * (c + 1), :] = (
                r["ypx"][u].astype(np.float32))
    acc = res.results[0]["ypl"].astype(np.float32).copy()
    for c in range(1, NCORES):
        acc += res.results[c]["ypl"].astype(np.float32)
    y[3] = acc
    # b_v folded here: y_ref = (attn + b_v) @ w_proj + b_proj
    y += np.asarray(b_proj, dtype=np.float32) + b_attn[2 * C:] @ w_proj
    return y


# revision 40
# speedup vs baseline: 1.0008x; 1.0008x over previous
"""Causal self-attention (GPT-style block) on 8 Trainium2 NeuronCores.

Sharding: tensor-parallel over heads (16 heads / 8 cores = 2 heads/core).
c_attn column-parallel from full x; attention fully local per core;
c_proj token-parallel after on-device AllToAll exchanges for batches
0..2, row-parallel for batch 3 (host sums the 8 partials).

Key structural choices (all matmuls contract over the partition dim;
matmul inputs are bf16 so every matmul runs at 1 cycle/row on the PE
regardless of free size):
- x is host-pretransposed/tiled to xp[tb, p, kt, s] (bf16) so stage 1
  needs no device transposes.
- q,k produced channel-major ([chan, tok]); v produced DIRECTLY
  token-major (stationary = x tile, moving = w_v) into vaug tiles
  [tok, v_h0(64) | 1 | v_h1(64) | 1]; the ones columns accumulate the
  softmax denominators inside the PV matmuls.
- b_q/b_k folded into the psum eviction (tensor_scalar_add); b_v is
  folded into b_proj ON THE HOST (b_v @ w_proj is a constant row).
- Scores computed transposed per 512-query block: S^T[key, q] with the
  2 heads packed in one [P, 2, 512] psum tile (row-tiled PE loads);
  exp is a single ACT instruction per key tile (strided AP covers both
  heads, including partial diagonal widths); causal mask applied
  multiplicatively to E on DVE.
- PV is TOKEN-major: out[q, chan] = sum_k E^T[k,q] v[k,chan] with
  stationary = 128x128 E tile, moving = vaug slice (65 free). K=128
  and out partitions = queries: half the PE cost of channel-major PV,
  and softmax normalization becomes a per-partition tensor_scalar_mul
  fused into the eviction. A PE transpose restores channel-major yT
  for the projection.
- c_proj: one AllToAll per batch 0..2 (the collective cost model has
  ~15us fixed overhead, so fewer+bigger wins); batch b's projection is
  emitted late in attention(b+1), hiding the collective latency.
- Stage 1 of batch b+1, ready projections, and b3's row-parallel
  partial projections are interleaved into attention's instruction
  stream via filler queues, so the PE never idles behind ACT's exp
  latency.
"""

import collections

import numpy as np

P = 128
B = 4
T = 2048
BT = B * T            # 8192 tokens
C = 1024
KT = C // P           # 8 contraction tiles of 128 input channels
NTB = BT // 512       # 16 token blocks of 512
HD = 64               # head dim
NQ = T // 512         # 4 query blocks per batch
NCORES = 8
TPC = 256             # tokens per core per exchanged batch

_CACHED = {}


def _build_nc():
    import concourse.mybir as mybir
    import concourse.tile as tile
    from concourse import bacc
    from concourse.masks import make_identity

    f32 = mybir.dt.float32
    bf16 = mybir.dt.bfloat16
    EXP = mybir.ActivationFunctionType.Exp

    nc = bacc.Bacc("TRN2", target_bir_lowering=False, debug=False,
                   num_devices=NCORES)

    xp = nc.dram_tensor("xp", [NTB, P, KT, 512], bf16, kind="ExternalInput")
    wq = nc.dram_tensor("wq", [P, KT, P], bf16, kind="ExternalInput")
    wk = nc.dram_tensor("wk", [P, KT, P], bf16, kind="ExternalInput")
    wv = nc.dram_tensor("wv", [P, KT, P], bf16, kind="ExternalInput")
    wp = nc.dram_tensor("wp", [P, KT, C], bf16, kind="ExternalInput")
    wpr = nc.dram_tensor("wpr", [P, C], bf16, kind="ExternalInput")
    bq = nc.dram_tensor("bq", [P, 1], f32, kind="ExternalInput")
    bk = nc.dram_tensor("bk", [P, 1], f32, kind="ExternalInput")
    # outputs: ypx[u] = batch u, this core's 256 tokens; ypl = batch 3
    # row-parallel partial over all 2048 tokens (host sums 8 cores)
    ypx = nc.dram_tensor("ypx", [3, TPC, C], bf16, kind="ExternalOutput")
    ypl = nc.dram_tensor("ypl", [T, C], bf16, kind="ExternalOutput")

    with tile.TileContext(nc) as tc:
        with (
            tc.tile_pool(name="const", bufs=1) as const,
            tc.tile_pool(name="xt", bufs=3) as xt_pool,
            tc.tile_pool(name="slab", bufs=2) as slab_pool,
            tc.tile_pool(name="e", bufs=4) as e_pool,
            tc.tile_pool(name="ytn", bufs=4) as ytn_pool,
            tc.tile_pool(name="nrm", bufs=8) as nrm_pool,
            tc.tile_pool(name="ob", bufs=4) as ob_pool,
            tc.tile_pool(name="yg", bufs=2) as yg_pool,
            tc.tile_pool(name="dram", bufs=1, space="DRAM") as dram_pool,
            tc.tile_pool(name="pss", bufs=2, space="PSUM") as pss_pool,
            tc.tile_pool(name="pvo", bufs=2, space="PSUM") as pvo_pool,
            tc.tile_pool(name="ps1", bufs=2, space="PSUM") as ps1_pool,
        ):
            g_in = [dram_pool.tile([NCORES, P, TPC], bf16,
                                   name=f"g_in{u}", tag=f"g_in{u}")
                    for u in range(3)]
            g_out = [dram_pool.tile([NCORES, P, TPC], bf16,
                                    name=f"g_out{u}", tag=f"g_out{u}")
                     for u in range(3)]

            # --- constants / weights resident in SBUF ---
            wq_sb = const.tile([P, KT, P], bf16)
            wk_sb = const.tile([P, KT, P], bf16)
            wv_sb = const.tile([P, KT, P], bf16)
            wp_sb = const.tile([P, KT, C], bf16)
            wpr_sb = const.tile([P, C], bf16)
            bq_sb = const.tile([P, 1], f32)
            bk_sb = const.tile([P, 1], f32)

            ident_f = const.tile([P, P], f32)
            make_identity(nc, ident_f[:])
            ident = const.tile([P, P], bf16)
            nc.vector.tensor_copy(ident[:], ident_f[:])

            # mask[p, s] = 1.0 if s >= p else 0.0 (keep q >= k)
            mask_f = const.tile([P, P], f32)
            nc.gpsimd.memset(mask_f[:], 1.0)
            nc.gpsimd.affine_select(
                out=mask_f[:],
                in_=mask_f[:],
                compare_op=mybir.AluOpType.is_ge,
                fill=0.0,
                base=0,
                pattern=[[1, P]],
                channel_multiplier=-1,
            )
            mask2 = const.tile([P, 2, P], bf16)
            nc.vector.tensor_copy(mask2[:, 0, :], mask_f[:])
            nc.vector.tensor_copy(mask2[:, 1, :], mask_f[:])

            # kt-0 slice first: the very first matmul only needs 32KB;
            # the bulk of wq follows the first x chunk (see load_xt)
            nc.sync.dma_start(wq_sb[:, 0:1, :], wq[:, 0:1, :])
            # w_proj loads deferred off the startup path; scalar queue so
            # they don't delay xp streaming on the sync queue
            wp_loaded = []

            def load_wp():
                if not wp_loaded:
                    # hold the 2MB load back so the scheduler can't hoist it
                    # over the startup xp chunks on the shared DMA engine
                    with tc.tile_wait_until(0.04):
                        nc.scalar.dma_start(wp_sb[:], wp[:])
                        nc.scalar.dma_start(wpr_sb[:], wpr[:])
                    wp_loaded.append(True)

            slabs = {}
            yTs = {}

            # ---------------- stage 1 (qkv) ----------------
            def stage1_units(b, startup=False):
                """Emission closures for batch b's qkv work, ordered so each
                x-block's DMA is issued ~4 units before its first use."""
                qT = slab_pool.tile([P, T], bf16, tag="qT", name=f"qT{b}")
                kT = slab_pool.tile([P, T], bf16, tag="kT", name=f"kT{b}")
                vaug = slab_pool.tile([P, T // P, 2 * HD + 2], bf16,
                                      tag="vaug", name=f"vaug{b}")
                slabs[b] = (qT, kT, vaug)

                def ones_cols():
                    nc.vector.memset(vaug[:, :, HD:HD + 1], 1.0)
                    nc.vector.memset(vaug[:, :, 2 * HD + 1:2 * HD + 2], 1.0)

                xts = {}

                def load_xt(lb, chunked=False):
                    def emit():
                        xt = xt_pool.tile([P, KT, 512], bf16, tag="xt",
                                          name=f"xt{b}_{lb}")
                        xts[lb] = xt
                        tb = b * NQ + lb
                        if chunked:
                            # front-load a tiny first chunk so the PE can
                            # start ~1us in; balance the rest against the
                            # per-DMA HWDGE overhead
                            nc.sync.dma_start(xt[:, 0:1, :], xp[tb, :, 0:1, :])
                            nc.sync.dma_start(wq_sb[:, 1:KT, :], wq[:, 1:KT, :])
                            nc.sync.dma_start(xt[:, 1:4, :], xp[tb, :, 1:4, :])
                            nc.sync.dma_start(wk_sb[:], wk[:])
                            nc.sync.dma_start(xt[:, 4:8, :], xp[tb, :, 4:8, :])
                            nc.sync.dma_start(wv_sb[:], wv[:])
                            nc.sync.dma_start(bq_sb[:], bq[:])
                            nc.sync.dma_start(bk_sb[:], bk[:])
                        else:
                            nc.sync.dma_start(xt[:], xp[tb])
                    return emit

                def qk_group(lb, w_sb, b_sb, dst):
                    def emit():
                        xt = xts[lb]
                        sl = slice(lb * 512, (lb + 1) * 512)
                        ps = ps1_pool.tile([P, 512], f32, tag="ps1",
                                           name=f"ps_qk{b}_{lb}")
                        for kt in range(KT):
                            nc.tensor.matmul(ps[:], w_sb[:, kt, :],
                                             xt[:, kt, :],
                                             start=(kt == 0),
                                             stop=(kt == KT - 1))
                        nc.vector.tensor_scalar_add(dst[:, sl], ps[:], b_sb[:])
                    return emit

                def v_group(lb, pair):
                    def emit():
                        xt = xts[lb]
                        psv = ps1_pool.tile([P, 512], f32, tag="ps1",
                                            name=f"ps_v{b}_{lb}_{pair}")
                        for t4 in (2 * pair, 2 * pair + 1):
                            off = t4 * P
                            tsl = slice(off, off + P)
                            for kt in range(KT):
                                nc.tensor.matmul(psv[:, tsl],
                                                 xt[:, kt, tsl],
                                                 wv_sb[:, kt, :],
                                                 start=(kt == 0),
                                                 stop=(kt == KT - 1))
                            j4 = lb * 4 + t4
                            nc.vector.tensor_copy(vaug[:, j4, 0:HD],
                                                  psv[:, off:off + HD])
                            nc.vector.tensor_copy(
                                vaug[:, j4, HD + 1:2 * HD + 1],
                                psv[:, off + HD:off + P])
                    return emit

                QK, VG = 1707, 854   # est PE ns per unit
                units = [(0, ones_cols),
                         (0, load_xt(0, chunked=startup)),
                         (QK, qk_group(0, wq_sb, bq_sb, qT)),
                         (0, load_xt(1)),
                         (QK, qk_group(0, wk_sb, bk_sb, kT)),
                         (VG, v_group(0, 0)), (VG, v_group(0, 1))]
                for lb in range(1, NQ):
                    units.append((QK, qk_group(lb, wq_sb, bq_sb, qT)))
                    if lb + 1 < NQ:
                        units.append((0, load_xt(lb + 1)))
                    units.append((QK, qk_group(lb, wk_sb, bk_sb, kT)))
                    units.append((VG, v_group(lb, 0)))
                    units.append((VG, v_group(lb, 1)))
                return units

            # ---------------- c_proj pieces ----------------
            def proj_unit(u, tok0, act_half=False):
                """One 128-token projection tile from exchanged batch u.
                act_half splits the eviction DVE/ACT -- only for units that
                run in batches where ACT is not the local metronome."""
                def emit():
                    yg = yg_tiles[u]
                    tsl = slice(tok0, tok0 + P)
                    pp0 = ps1_pool.tile([P, 512], f32, tag="ps1",
                                        name=f"pp0_{u}_{tok0}")
                    pp1 = ps1_pool.tile([P, 512], f32, tag="ps1",
                                        name=f"pp1_{u}_{tok0}")
                    for ct in range(KT):
                        nc.tensor.matmul(pp0[:], yg[:, ct, tsl],
                                         wp_sb[:, ct, 0:512],
                                         start=(ct == 0), stop=(ct == KT - 1))
                    for ct in range(KT):
                        nc.tensor.matmul(pp1[:], yg[:, ct, tsl],
                                         wp_sb[:, ct, 512:C],
                                         start=(ct == 0), stop=(ct == KT - 1))
                    ob = ob_pool.tile([P, C], bf16, tag="ob",
                                      name=f"ob_{u}_{tok0}")
                    nc.vector.tensor_copy(ob[:, 0:512], pp0[:])
                    if act_half:
                        nc.scalar.copy(ob[:, 512:C], pp1[:])
                    else:
                        nc.vector.tensor_copy(ob[:, 512:C], pp1[:])
                    out_dmas.append(
                        lambda: nc.scalar.dma_start(ypx[u, tsl, :], ob[:]))
                return emit

            def partial_unit(pt, tail=False):
                """Row-parallel partial proj for b3 tokens [128pt, +128)."""
                def emit():
                    yT3 = yTs[3]
                    ssl = slice(pt * P, (pt + 1) * P)
                    pp0 = ps1_pool.tile([P, 512], f32, tag="ps1",
                                        name=f"ppl0_{pt}")
                    pp1 = ps1_pool.tile([P, 512], f32, tag="ps1",
                                        name=f"ppl1_{pt}")
                    nc.tensor.matmul(pp0[:], yT3[:, ssl], wpr_sb[:, 0:512],
                                     start=True, stop=True)
                    nc.tensor.matmul(pp1[:], yT3[:, ssl], wpr_sb[:, 512:C],
                                     start=True, stop=True)
                    ob = ob_pool.tile([P, C], bf16, tag="ob",
                                      name=f"obl_{pt}")
                    nc.vector.tensor_copy(ob[:, 0:512], pp0[:])
                    if tail:
                        # ACT is idle at the drain; halve the DVE chain
                        nc.scalar.copy(ob[:, 512:C], pp1[:])
                    else:
                        nc.vector.tensor_copy(ob[:, 512:C], pp1[:])
                    out_dmas.append(
                        lambda: nc.scalar.dma_start(ypl[ssl, :], ob[:]))
                return emit

            yg_tiles = {}
            out_dmas = collections.deque()
            deferred = collections.deque()  # norm->transpose + exchanges

            def emit_exchange(u):
                """AllToAll batch u's yT; peer j gets this core's 2
                head-channels for peer j's 256 tokens."""
                yT_ = yTs[u]
                for j in range(NCORES):
                    # gpsimd queue: these wait on late yT regions; on the SP
                    # queue that wait would block the next batch's xt loads
                    nc.gpsimd.dma_start(g_in[u][j],
                                        yT_[:, j * TPC:(j + 1) * TPC])
                nc.gpsimd.collective_compute(
                    "AllToAll",
                    mybir.AluOpType.bypass,
                    replica_groups=[list(range(NCORES))],
                    ins=[g_in[u][:]],
                    outs=[g_out[u][:]],
                )
                # gather on the gpsimd queue: it naturally orders after the
                # collective without blocking any busy engine's queue
                yg = yg_pool.tile([P, NCORES, TPC], bf16, tag="yg",
                                  name=f"yg{u}")
                nc.gpsimd.dma_start(yg[:], g_out[u].rearrange("c p t -> p c t"))
                yg_tiles[u] = yg

            # ---------------- attention ----------------
            def attention(b, fillers, late2=(), late3=(), own=()):
                qT, kT, vaug = slabs[b]
                yT = slab_pool.tile([P, T], bf16, tag="yT", name=f"yT{b}")
                yTs[b] = yT
                fillers = collections.deque(fillers)
                late2 = collections.deque(late2)
                late3 = collections.deque(late3)
                own = collections.deque(own)  # (block, cost, closure)
                need = 0.0   # cumulative ACT-minus-PE ns not yet filled

                def pump(i, j_done, j_total):
                    nonlocal need
                    if deferred:
                        deferred.popleft()()
                    if out_dmas:
                        out_dmas.popleft()()
                    if own:
                        own.popleft()[2]()
                        return
                    if i >= 2 and late2 and (j_done % 2 == 0):
                        late2.popleft()[1]()
                        return
                    if i >= 3 and late3 and (j_done % 2 == 1):
                        late3.popleft()[1]()
                        return
                    rem_j = j_total - j_done
                    while fillers and len(fillers) >= rem_j:
                        fillers.popleft()[1]()
                    if fillers and (j_done % 2 == 0):
                        fillers.popleft()[1]()

                j_total = sum(4 * (i + 1) for i in range(NQ))
                j_done = 0
                for i in range(NQ):
                    # this batch's own stage-1 blocks <= i must be emitted
                    # before block i's scores touch them
                    while own and own[0][0] <= i:
                        _, cost, u = own.popleft()
                        u()
                        need -= max(cost, 400)
                    nj = 4 * (i + 1)
                    # 8 accumulation regions share 2 banks. The FIRST matmul
                    # into each bank (j=0, lqt 0/2, h0) uses start=True: its
                    # whole-zero-region lazy-clear zeroes all 4 regions of
                    # the bank at once; every other matmul accumulates.
                    pvo_a = pvo_pool.tile([P, 512], f32, tag="pvo",
                                          name=f"pvo_a{b}_{i}")
                    pvo_b = pvo_pool.tile([P, 512], f32, tag="pvo",
                                          name=f"pvo_b{b}_{i}")
                    pvs = (pvo_a, pvo_b)

                    def emit_s(j):
                        q0 = max(0, (j - 4 * i)) * P
                        jsl = slice(j * P, (j + 1) * P)
                        qsl = slice(i * 512 + q0, (i + 1) * 512)
                        psp = pss_pool.tile([P, 2, 512], f32, tag="pss",
                                            name=f"psp{b}_{i}_{j}")
                        nc.tensor.matmul(psp[:, 0, q0:512], kT[0:HD, jsl],
                                         qT[0:HD, qsl], start=True, stop=True,
                                         tile_position=(0, 0))
                        nc.tensor.matmul(psp[:, 1, q0:512], kT[HD:P, jsl],
                                         qT[HD:P, qsl], start=True, stop=True,
                                         tile_position=(HD, 0))
                        return psp

                    def emit_e(j, psp):
                        q0 = max(0, (j - 4 * i)) * P
                        ep = e_pool.tile([P, 2, 512], bf16, tag="e",
                                         name=f"ep{b}_{i}_{j}")
                        # one activation covers both heads, including the
                        # partial diagonal widths (strided [2, w] AP)
                        nc.scalar.activation(ep[:, :, q0:512],
                                             psp[:, :, q0:512], EXP,
                                             scale=0.125)
                        if j - 4 * i >= 0:
                            d = j - 4 * i
                            msl = slice(d * P, (d + 1) * P)
                            nc.vector.tensor_mul(ep[:, 0, msl], ep[:, 0, msl],
                                                 mask2[:, 0, :])
                            nc.vector.tensor_mul(ep[:, 1, msl], ep[:, 1, msl],
                                                 mask2[:, 1, :])
                        return ep

                    def emit_norm(b_, qt, psA, off):
                        rinv = nrm_pool.tile([P, 2], f32, tag="rinv",
                                             name=f"rinv{b_}_{qt}")
                        nc.vector.reciprocal(rinv[:, 0:1],
                                             psA[:, off + HD:off + HD + 1])
                        nc.vector.reciprocal(rinv[:, 1:2],
                                             psA[:, off + 129:off + 130])
                        ytn = ytn_pool.tile([P, P], bf16, tag="ytn",
                                            name=f"ytn{b_}_{qt}")
                        nc.vector.tensor_scalar_mul(
                            ytn[:, 0:HD], psA[:, off:off + HD], rinv[:, 0:1])
                        nc.vector.tensor_scalar_mul(
                            ytn[:, HD:P], psA[:, off + HD + 1:off + 129],
                            rinv[:, 1:2])

                        def finish():
                            psT = ps1_pool.tile([P, P], bf16, tag="ps1",
                                                name=f"psT{b_}_{qt}")
                            nc.tensor.transpose(psT[:], ytn[:], ident[:])
                            nc.vector.tensor_copy(
                                yT[:, qt * P:(qt + 1) * P], psT[:])
                        deferred.append(finish)

                    def emit_pv(j, ep):
                        d = max(0, j - 4 * i)
                        for lqt in range(d, 4):
                            qt = 4 * i + lqt
                            psA = pvs[lqt // 2]
                            off = (lqt % 2) * 256
                            esl = slice(lqt * P, (lqt + 1) * P)
                            sp = (j == qt)
                            st = (j == 0 and lqt in (0, 2))
                            nc.tensor.matmul(psA[:, off:off + HD + 1],
                                             ep[:, 0, esl],
                                             vaug[:, j, 0:HD + 1],
                                             start=st, stop=sp,
                                             skip_group_check=True)
                            nc.tensor.matmul(psA[:, off + HD + 1:off + 130],
                                             ep[:, 1, esl],
                                             vaug[:, j, HD + 1:2 * HD + 2],
                                             start=False, stop=sp,
                                             skip_group_check=True)
                            if sp:
                                emit_norm(b, qt, psA, off)

                    # software pipeline: S(j+1) issued before PV(j) so the
                    # PE never sits directly behind ACT's exp latency
                    psps = {0: emit_s(0)}
                    eps = {}
                    for j in range(nj):
                        eps[j] = emit_e(j, psps.pop(j))
                        if j + 1 < nj:
                            psps[j + 1] = emit_s(j + 1)
                        emit_pv(j, eps.pop(j))
                        j_done += 1
                        pump(i, j_done, j_total)
                while own:
                    own.popleft()[2]()
                for q in (late2, late3, fillers):
                    while q:
                        q.popleft()[1]()

            # ---------------- main schedule ----------------
            # Per-phase PE load balancing: each batch's attention carries
            # its OWN stage-1 blocks 1-3 (block i forced before query-block
            # i), while only block 0 of the NEXT batch is prefed late in the
            # current attention. This levels PE work across all 4 phases.
            BLK = [1, 1, 1, 1, 1, 2, 2, 2, 2, 2, 3, 3, 3, 3]

            def split_units(units):
                # pre: block 0 (7 units), own: blocks 1-3 folded into the
                # batch's own attention
                return (units[:7],
                        [(bi, c, u) for bi, (c, u) in zip(BLK, units[7:])])

            s0 = stage1_units(0, startup=True)
            pre0, own0 = split_units(s0)
            for _, u in pre0:
                u()

            PJ, PL = 3414, 854
            pre1, own1 = split_units(stage1_units(1))
            attention(0, [], late3=list(pre1), own=own0)
            deferred.append(lambda: emit_exchange(0))
            load_wp()

            pre2, own2 = split_units(stage1_units(2))
            attention(1, [], own=own1,
                      late3=[(PJ, proj_unit(0, 0, act_half=True))] + list(pre2))
            deferred.append(lambda: emit_exchange(1))

            # batch-0/1 proj units are ready well before b2/b3 start, so
            # they serve as base fillers there instead of crowding late3
            pre3, own3 = split_units(stage1_units(3))
            attention(2, [(PJ, proj_unit(0, 128, act_half=True))], own=own2,
                      late3=[(PJ, proj_unit(1, 0, act_half=True))] + list(pre3))
            deferred.append(lambda: emit_exchange(2))

            # b3: batch-1/2 proj + row-parallel partials as fillers
            late2 = [(PL, partial_unit(pt)) for pt in range(8)]
            late3 = ([(PL, partial_unit(pt)) for pt in range(8, 12)]
                     + [(PJ, proj_unit(2, 0)), (PJ, proj_unit(2, 128))]
                     + [(PL, partial_unit(12)), (PL, partial_unit(13))])
            attention(3, [(PJ, proj_unit(1, 128))], late2=late2, late3=late3,
                      own=own3)

            # tail: last partial tiles (tokens 1536:2048 of b3)
            while deferred:
                deferred.popleft()()
            for pt in range(14, 16):
                partial_unit(pt, tail=True)()
                while out_dmas:
                    out_dmas.popleft()()

    nc.compile()
    return nc


def _prep_inputs(x, w_attn, b_attn, w_proj):
    import ml_dtypes
    bf16 = ml_dtypes.bfloat16

    x = np.asarray(x, dtype=np.float32)
    w_attn = np.asarray(w_attn, dtype=np.float32)
    b_attn = np.asarray(b_attn, dtype=np.float32)
    w_proj = np.asarray(w_proj, dtype=np.float32)

    x_flat = x.reshape(BT, C)
    # xp[tb, p, kt, s] = x_flat[tb*512+s, kt*128+p]
    xp = np.ascontiguousarray(
        x_flat.T.reshape(KT, P, NTB, 512).transpose(2, 1, 0, 3)).astype(bf16)

    wp = np.ascontiguousarray(
        w_proj.reshape(KT, P, C).transpose(1, 0, 2)).astype(bf16)
    in_maps = []
    for c in range(NCORES):
        cols = slice(P * c, P * (c + 1))

        def wslice(off):
            w = w_attn[:, off + P * c: off + P * (c + 1)]   # [1024, 128]
            return np.ascontiguousarray(
                w.reshape(KT, P, P).transpose(1, 0, 2)).astype(bf16)

        in_maps.append({
            "xp": xp,
            "wq": wslice(0),
            "wk": wslice(C),
            "wv": wslice(2 * C),
            "wp": wp,
            "wpr": np.ascontiguousarray(w_proj[cols, :]).astype(bf16),
            "bq": np.ascontiguousarray(b_attn[cols]).reshape(P, 1),
            "bk": np.ascontiguousarray(
                b_attn[C + P * c: C + P * (c + 1)]).reshape(P, 1),
        })
    return in_maps


def kernel(x, w_attn, b_attn, w_proj, b_proj):
    from concourse.bass_utils import run_bass_kernel_spmd

    if "nc" not in _CACHED:
        _CACHED["nc"] = _build_nc()
    nc = _CACHED["nc"]

    in_maps = _prep_inputs(x, w_attn, b_attn, w_proj)
    res = run_bass_kernel_spmd(nc, in_maps, core_ids=list(range(NCORES)))

    w_proj = np.asarray(w_proj, dtype=np.float32)
    b_attn = np.asarray(b_attn, dtype=np.float32)
    y = np.empty((B, T, C), dtype=np.float32)
    for c in range(NCORES):
        r = res.results[c]
        for u in range(3):
            y[u, TPC * c:TPC * (c + 1), :] = (
                r["ypx"][u].astype(np.float32))
    acc = res.results[0]["ypl"].astype(np.float32).copy()
    for c in range(1, NCORES):
        acc += res.results[c]["ypl"].astype(np.float32)
    y[3] = acc
    # b_v folded here: y_ref = (attn + b_v) @ w_proj + b_proj
    y += np.asarray(b_proj, dtype=np.float32) + b_attn[2 * C:] @ w_proj
    return y
